# revision 10
# baseline (speedup 1.0000x reference)
"""CrissCrossAttention (full HW-token attention) Trainium2 kernel.

Reference computation (B=4, C=256, H=W=64, N=H*W=4096, CQK=32):
    q = wq@x+bq   [B,32,N]
    k = wk@x+bk   [B,32,N]
    v = wv@x+bv   [B,256,N]
    energy = q^T k      [B,N,N]
    attn = softmax_j(energy)
    out = v @ attn^T    [B,256,N]
    final = x + wg@out + bg

Sharding: 8 cores = 4 batches x 2 query-halves. Each core receives x[b]
rolled so its 2048 query columns are always columns 0:2048 (softmax over
keys is permutation invariant, so rolling keys+values consistently leaves
the result unchanged) -> one identical SPMD program for all cores.

Algebraic folding done on host:
    wg@(v@attn^T)+bg = (wg@wv)@x@attn^T + (wg@bv+bg)    (attn rows sum to 1)
so the kernel only needs W=wg@wv and b=wg@bv+bg, and the g-projection
matmul disappears.

Device layout trick: energy tiles are computed transposed, S_t[j,i]=k^T q,
so the exp'd tile P_t[j,i] feeds the AV matmul directly as the stationary
operand (no transposes anywhere in the main loop). Ones-columns appended
to vW_t produce the softmax denominator inside the same accumulation.
All matmul operands are float32r (FP22-truncated reads, full PE rate).
"""

import sys

import numpy as np

_B, _C, _H, _W = 4, 256, 64, 64
_N = _H * _W  # 4096 key/value positions
_CQK = _C // 8  # 32
_NCORES = 8
_NQ = _N // 2  # 2048 queries per core

# Filled by kernel() for the benefit of test harnesses; never read here.
LAST_RUN_INFO = {}
TRACE = False

_REPO = "/opt/trn_rl_repo"


def _ensure_path():
    if _REPO not in sys.path:
        sys.path.insert(0, _REPO)


def build_program(n=_N, nq=_NQ, jpb=2, reps=1):
    """Build the single-core Bass/Tile program (identical across cores).

    n:    number of key/value positions    (multiple of 512)
    nq:   number of query positions        (multiple of 512)
    jpb:  key j-subtiles (128 keys each) batched per PSUM/exp tile
    reps: repeat the compute body in a HW loop (benchmarking only)
    """
    _ensure_path()
    import concourse.tile as tile
    from concourse import bacc, mybir
    from concourse.bass import ds, ts

    f32 = mybir.dt.float32
    f32r = mybir.dt.float32r
    Exp = mybir.ActivationFunctionType.Exp
    mult = mybir.AluOpType.mult
    add = mybir.AluOpType.add

    P = 128
    IW = 512  # query-tile width for the energy matmul (N of one MM)
    assert n % (128 * jpb) == 0 and nq % IW == 0
    NJ = n // 128  # j-tiles of 128 keys
    NJB = NJ // jpb  # j batches
    NI = nq // IW  # i-tiles of 512 queries
    NSL = IW // P  # 4 i-slices per i-tile

    nc = bacc.Bacc("TRN2", target_bir_lowering=False, debug=False)

    x_in = nc.dram_tensor("x_in", [_C, n], f32r, kind="ExternalInput")
    xqt_b = nc.dram_tensor("xqt_b", [nq, _C], f32, kind="ExternalInput")
    wq4t = nc.dram_tensor("wq4t", [_C, 128], f32r, kind="ExternalInput")
    wk4t = nc.dram_tensor("wk4t", [_C, 128], f32r, kind="ExternalInput")
    bq4 = nc.dram_tensor("bq4", [128, 1], f32, kind="ExternalInput")
    bk4 = nc.dram_tensor("bk4", [128, 1], f32, kind="ExternalInput")
    Wt = nc.dram_tensor("Wt", [_C, _C], f32r, kind="ExternalInput")
    ones_col = nc.dram_tensor("ones_col", [128, NJ, 4], f32r, kind="ExternalInput")
    out_t = nc.dram_tensor("out_t", [nq, _C], f32, kind="ExternalOutput")

    with tile.TileContext(nc) as tc:
        with (
            tc.tile_pool(name="singles", bufs=1) as singles,
            tc.tile_pool(name="ptile", bufs=3) as ppool,
            tc.tile_pool(name="epi", bufs=4) as epool,
            tc.tile_pool(name="xq", bufs=4) as xqpool,
            tc.tile_pool(name="spsum", bufs=2, space="PSUM") as spool,
            tc.tile_pool(name="accpsum", bufs=4, space="PSUM") as accpool,
        ):
            # ---- persistent SBUF tensors ----
            x_sb = [
                singles.tile([P, n], f32r, tag=f"x{c}", name=f"x_sb{c}")
                for c in range(2)
            ]
            k4_sb = singles.tile([P, n], f32r, tag="k4")
            q4_sb = singles.tile([P, nq], f32r, tag="q4")
            vW1_sb = singles.tile([P, NJ, 260], f32r, tag="vw1")
            wq4_sb = [
                singles.tile([P, 128], f32r, tag=f"wq{c}", name=f"wq4_sb{c}")
                for c in range(2)
            ]
            wk4_sb = [
                singles.tile([P, 128], f32r, tag=f"wk{c}", name=f"wk4_sb{c}")
                for c in range(2)
            ]
            Wt_sb = [
                singles.tile([P, _C], f32r, tag=f"wt{c}", name=f"Wt_sb{c}")
                for c in range(2)
            ]
            bq4_sb = singles.tile([P, 1], f32, tag="bq")
            bk4_sb = singles.tile([P, 1], f32, tag="bk")

            for c in range(2):
                nc.sync.dma_start(out=x_sb[c], in_=x_in[c * P : (c + 1) * P, :])
                nc.sync.dma_start(out=wq4_sb[c], in_=wq4t[c * P : (c + 1) * P, :])
                nc.sync.dma_start(out=wk4_sb[c], in_=wk4t[c * P : (c + 1) * P, :])
                nc.sync.dma_start(out=Wt_sb[c], in_=Wt[c * P : (c + 1) * P, :])
            nc.sync.dma_start(out=bq4_sb, in_=bq4[:, :])
            nc.sync.dma_start(out=bk4_sb, in_=bk4[:, :])

            # ones columns -> softmax denominator rides along the AV matmul
            nc.sync.dma_start(out=vW1_sb[:, :, 256:260], in_=ones_col[:, :, :])

            def compute_body():
                # ---- projections ----
                # k (4x replicated over partition groups): k4 = wk4t^T @ x + bk
                for t in range(n // IW):
                    kp = spool.tile([P, IW], f32, tag="s", name="kp")
                    for c in range(2):
                        nc.tensor.matmul(
                            kp,
                            wk4_sb[c][:, :],
                            x_sb[c][:, ts(t, IW)],
                            start=(c == 0),
                            stop=(c == 1),
                        )
                    nc.vector.tensor_scalar_add(k4_sb[:, ts(t, IW)], kp, bk4_sb[:, :])

                # q for our query columns (0:nq of the rolled x)
                for t in range(nq // IW):
                    qp = spool.tile([P, IW], f32, tag="s", name="qp")
                    for c in range(2):
                        nc.tensor.matmul(
                            qp,
                            wq4_sb[c][:, :],
                            x_sb[c][:, ts(t, IW)],
                            start=(c == 0),
                            stop=(c == 1),
                        )
                    nc.vector.tensor_scalar_add(q4_sb[:, ts(t, IW)], qp, bq4_sb[:, :])

                # vW_t[j, c] = (W @ x)^T = x^T @ W^T, per j-tile
                for j in range(NJ):
                    vp = spool.tile([P, _C], f32, tag="s", name="vp")
                    for c in range(2):
                        nc.tensor.matmul(
                            vp,
                            x_sb[c][:, ts(j, P)],
                            Wt_sb[c][:, :],
                            start=(c == 0),
                            stop=(c == 1),
                        )
                    nc.vector.tensor_copy(vW1_sb[:, j, 0:256], vp)

                # ---- attention main loop ----
                for i in range(NI):
                    accs = [
                        accpool.tile([P, 260], f32, tag="acc", name="acc")
                        for _ in range(NSL)
                    ]
                    for jb in range(NJB):
                        sp = spool.tile([P, jpb * IW], f32, tag="s", name="sp")
                        for t in range(jpb):
                            jt = jb * jpb + t
                            # S_t[j, i] = sum_d k[d, j] * q[d, i]   (K = 32)
                            nc.tensor.matmul(
                                sp[:, ts(t, IW)],
                                k4_sb[0:_CQK, ts(jt, P)],
                                q4_sb[0:_CQK, ts(i, IW)],
                                start=True,
                                stop=True,
                            )
                        pt = ppool.tile([P, jpb * IW], f32r, tag="p", name="pt")
                        nc.scalar.activation(pt, sp, Exp)
                        for t in range(jpb):
                            jt = jb * jpb + t
                            for s in range(NSL):
                                nc.tensor.matmul(
                                    accs[s],
                                    pt[:, ds(t * IW + s * P, P)],
                                    vW1_sb[:, jt, :],
                                    start=(jb == 0 and t == 0),
                                    stop=(jb == NJB - 1 and t == jpb - 1),
                                )
                    # epilogue: out = acc * (1/denom) + (x^T + b)
                    for s in range(NSL):
                        isl = i * IW + s * P
                        xq = xqpool.tile([P, _C], f32, tag="xq", name="xq")
                        nc.sync.dma_start(out=xq, in_=xqt_b[isl : isl + P, :])
                        rc = epool.tile([P, 1], f32, tag="rc", name="rc")
                        nc.vector.reciprocal(rc, accs[s][:, 256:257])
                        st = epool.tile([P, _C], f32, tag="st", name="st")
                        nc.vector.scalar_tensor_tensor(
                            st, accs[s][:, 0:256], rc[:, :], xq, op0=mult, op1=add
                        )
                        nc.sync.dma_start(out=out_t[isl : isl + P, :], in_=st)

            if reps > 1:
                with tc.For_i(0, reps, 1, hint_engines=(mybir.EngineType.PE,)):
                    compute_body()
            else:
                compute_body()

    nc.compile()
    return nc


def _host_inputs(x, wq, bq, wk, bk, wv, bv, wg, bg, n=_N, nq=_NQ):
    """Per-core input maps (numpy only)."""
    xf = np.ascontiguousarray(x.reshape(_B, _C, n).astype(np.float32))
    W64 = wg.astype(np.float64) @ wv.astype(np.float64)
    b64 = wg.astype(np.float64) @ bv.astype(np.float64) + bg.astype(np.float64)
    Wt = np.ascontiguousarray(W64.T.astype(np.float32))
    bcomb = b64.astype(np.float32)
    wq4t = np.ascontiguousarray(np.tile(wq.T.astype(np.float32), (1, 4)))
    wk4t = np.ascontiguousarray(np.tile(wk.T.astype(np.float32), (1, 4)))
    bq4 = np.ascontiguousarray(np.tile(bq.astype(np.float32), 4)[:, None])
    bk4 = np.ascontiguousarray(np.tile(bk.astype(np.float32), 4)[:, None])
    ones_col = np.ones((128, n // 128, 4), np.float32)

    halves = n // nq
    in_maps = []
    for core in range(_NCORES):
        b, half = core // halves, core % halves
        off = half * nq
        x_roll = np.ascontiguousarray(np.roll(xf[b], -off, axis=1))
        xqt_b = np.ascontiguousarray(x_roll[:, :nq].T + bcomb[None, :])
        in_maps.append(
            {
                "x_in": x_roll,
                "xqt_b": xqt_b,
                "wq4t": wq4t,
                "wk4t": wk4t,
                "bq4": bq4,
                "bk4": bk4,
                "Wt": Wt,
                "ones_col": ones_col,
            }
        )
    return in_maps


def kernel(x, wq, bq, wk, bk, wv, bv, wg, bg):
    _ensure_path()
    from concourse.bass_utils import run_bass_kernel_spmd

    nc = build_program()
    in_maps = _host_inputs(x, wq, bq, wk, bk, wv, bv, wg, bg)
    core_ids = list(range(_NCORES))
    res = run_bass_kernel_spmd(nc, in_maps, core_ids, trace=TRACE)
    LAST_RUN_INFO["exec_time_ns"] = res.exec_time_ns
    LAST_RUN_INFO["mean_exec_time_ns"] = res.mean_exec_time_ns
    LAST_RUN_INFO["results"] = res

    out = np.empty((_B, _C, _N), np.float32)
    for core in range(_NCORES):
        b, off = core // 2, (core % 2) * _NQ
        out[b, :, off : off + _NQ] = res.results[core]["out_t"].T
    return out.reshape(_B, _C, _H, _W)


# revision 16
# speedup vs baseline: 1.6431x; 1.6431x over previous
"""CrissCrossAttention (full HW-token attention) Trainium2 kernel.

Reference computation (B=4, C=256, H=W=64, N=H*W=4096, CQK=32):
    q = wq@x+bq   [B,32,N]
    k = wk@x+bk   [B,32,N]
    v = wv@x+bv   [B,256,N]
    energy = q^T k      [B,N,N]
    attn = softmax_j(energy)
    out = v @ attn^T    [B,256,N]
    final = x + wg@out + bg

Sharding: 8 cores = 4 batches x 2 query-halves. Each core receives x[b]
rolled so its 2048 query columns are always columns 0:2048 (softmax over
keys is permutation invariant, so rolling keys+values consistently leaves
the result unchanged) -> one identical SPMD program for all cores.

Algebraic folding done on host:
    wg@(v@attn^T)+bg = (wg@wv)@x@attn^T + (wg@bv+bg)    (attn rows sum to 1)
so the kernel only needs W=wg@wv and b=wg@bv+bg, and the g-projection
matmul disappears.

Device layout trick: energy tiles are computed transposed, S_t[j,i]=k^T q,
so the exp'd tile P_t[j,i] feeds the AV matmul directly as the stationary
operand (no transposes anywhere in the main loop). Ones-columns appended
to vW_t produce the softmax denominator inside the same accumulation.
All matmul operands are float32r (FP22-truncated reads, full PE rate).
"""

import sys

import numpy as np

_B, _C, _H, _W = 4, 256, 64, 64
_N = _H * _W  # 4096 key/value positions
_CQK = _C // 8  # 32
_NCORES = 8
_NQ = _N // 2  # 2048 queries per core

# Filled by kernel() for the benefit of test harnesses; never read here.
LAST_RUN_INFO = {}
TRACE = False

_REPO = "/opt/trn_rl_repo"


def _ensure_path():
    if _REPO not in sys.path:
        sys.path.insert(0, _REPO)


def build_program(n=_N, nq=_NQ, jpb=2, reps=1):
    """Build the single-core Bass/Tile program (identical across cores).

    n:    number of key/value positions    (multiple of 512)
    nq:   number of query positions        (multiple of 512)
    jpb:  key j-subtiles (128 keys each) batched per PSUM/exp tile
    reps: repeat the compute body in a HW loop (benchmarking only)
    """
    _ensure_path()
    import concourse.tile as tile
    from concourse import bacc, mybir
    from concourse.bass import ds, ts

    f32 = mybir.dt.float32
    f32r = mybir.dt.float32r
    bf16 = mybir.dt.bfloat16
    Exp = mybir.ActivationFunctionType.Exp
    mult = mybir.AluOpType.mult
    add = mybir.AluOpType.add

    P = 128
    IW = 512  # query-tile width for the energy matmul (N of one MM)
    assert n % (128 * jpb) == 0 and nq % IW == 0
    NJ = n // 128  # j-tiles of 128 keys
    NJB = NJ // jpb  # j batches
    NI = nq // IW  # i-tiles of 512 queries
    NSL = IW // P  # 4 i-slices per i-tile

    nc = bacc.Bacc("TRN2", target_bir_lowering=False, debug=False)

    x_in = nc.dram_tensor("x_in", [_C, n], f32r, kind="ExternalInput")
    xqt_b = nc.dram_tensor("xqt_b", [nq, _C], f32, kind="ExternalInput")
    wq4t = nc.dram_tensor("wq4t", [_C, 128], f32r, kind="ExternalInput")
    wk4t = nc.dram_tensor("wk4t", [_C, 128], f32r, kind="ExternalInput")
    bq4 = nc.dram_tensor("bq4", [128, 1], f32, kind="ExternalInput")
    bk4 = nc.dram_tensor("bk4", [128, 1], f32, kind="ExternalInput")
    Wt = nc.dram_tensor("Wt", [_C, _C], f32r, kind="ExternalInput")
    ones_col = nc.dram_tensor("ones_col", [128, NJ, 8], bf16, kind="ExternalInput")
    out_t = nc.dram_tensor("out_t", [nq, _C], f32, kind="ExternalOutput")

    with tile.TileContext(nc) as tc:
        with (
            tc.tile_pool(name="singles", bufs=1) as singles,
            tc.tile_pool(name="ptile", bufs=3) as ppool,
            tc.tile_pool(name="epi", bufs=4) as epool,
            tc.tile_pool(name="xq", bufs=4) as xqpool,
            tc.tile_pool(name="spsum", bufs=2, space="PSUM") as spool,
            tc.tile_pool(name="accpsum", bufs=4, space="PSUM") as accpool,
        ):
            # ---- persistent SBUF tensors ----
            x_sb = [
                singles.tile([P, n], f32r, tag=f"x{c}", name=f"x_sb{c}")
                for c in range(2)
            ]
            k4_sb = singles.tile([P, n], f32r, tag="k4")
            q4_sb = singles.tile([P, nq], f32r, tag="q4")
            vW1_sb = singles.tile([P, NJ, 264], bf16, tag="vw1")
            wq4_sb = [
                singles.tile([P, 128], f32r, tag=f"wq{c}", name=f"wq4_sb{c}")
                for c in range(2)
            ]
            wk4_sb = [
                singles.tile([P, 128], f32r, tag=f"wk{c}", name=f"wk4_sb{c}")
                for c in range(2)
            ]
            Wt_sb = [
                singles.tile([P, _C], f32r, tag=f"wt{c}", name=f"Wt_sb{c}")
                for c in range(2)
            ]
            bq4_sb = singles.tile([P, 1], f32, tag="bq")
            bk4_sb = singles.tile([P, 1], f32, tag="bk")

            for c in range(2):
                nc.sync.dma_start(out=x_sb[c], in_=x_in[c * P : (c + 1) * P, :])
                nc.sync.dma_start(out=wq4_sb[c], in_=wq4t[c * P : (c + 1) * P, :])
                nc.sync.dma_start(out=wk4_sb[c], in_=wk4t[c * P : (c + 1) * P, :])
                nc.sync.dma_start(out=Wt_sb[c], in_=Wt[c * P : (c + 1) * P, :])
            nc.sync.dma_start(out=bq4_sb, in_=bq4[:, :])
            nc.sync.dma_start(out=bk4_sb, in_=bk4[:, :])

            # ones columns -> softmax denominator rides along the AV matmul
            nc.sync.dma_start(out=vW1_sb[:, :, 256:264], in_=ones_col[:, :, :])

            def compute_body():
                # ---- projections ----
                # k (4x replicated over partition groups): k4 = wk4t^T @ x + bk
                for t in range(n // IW):
                    kp = spool.tile([P, IW], f32, tag="s", name="kp")
                    for c in range(2):
                        nc.tensor.matmul(
                            kp,
                            wk4_sb[c][:, :],
                            x_sb[c][:, ts(t, IW)],
                            start=(c == 0),
                            stop=(c == 1),
                        )
                    nc.vector.tensor_scalar_add(k4_sb[:, ts(t, IW)], kp, bk4_sb[:, :])

                # q for our query columns (0:nq of the rolled x)
                for t in range(nq // IW):
                    qp = spool.tile([P, IW], f32, tag="s", name="qp")
                    for c in range(2):
                        nc.tensor.matmul(
                            qp,
                            wq4_sb[c][:, :],
                            x_sb[c][:, ts(t, IW)],
                            start=(c == 0),
                            stop=(c == 1),
                        )
                    nc.vector.tensor_scalar_add(q4_sb[:, ts(t, IW)], qp, bq4_sb[:, :])

                # vW_t[j, c] = (W @ x)^T = x^T @ W^T, per j-tile
                for j in range(NJ):
                    vp = spool.tile([P, _C], f32, tag="s", name="vp")
                    for c in range(2):
                        nc.tensor.matmul(
                            vp,
                            x_sb[c][:, ts(j, P)],
                            Wt_sb[c][:, :],
                            start=(c == 0),
                            stop=(c == 1),
                        )
                    nc.vector.tensor_copy(vW1_sb[:, j, 0:256], vp)

                # ---- attention main loop ----
                for i in range(NI):
                    accs = [
                        accpool.tile([P, 264], f32, tag="acc", name="acc")
                        for _ in range(NSL)
                    ]

                    def emit_energy(jb, i=i):
                        # S_t[j, i] = sum_d k[d, j] * q[d, i]  (K = 32); the
                        # jpb j-subtiles go to distinct PE row groups (k/q
                        # are replicated across partition groups for this)
                        sp = spool.tile([P, jpb * IW], f32, tag="s", name="sp")
                        for t in range(jpb):
                            jt = jb * jpb + t
                            nc.tensor.matmul(
                                sp[:, ts(t, IW)],
                                k4_sb[32 * t : 32 * t + _CQK, ts(jt, P)],
                                q4_sb[32 * t : 32 * t + _CQK, ts(i, IW)],
                                start=True,
                                stop=True,
                                tile_position=(32 * t, 0),
                            )
                        return sp

                    sps = {0: emit_energy(0)}
                    for jb in range(NJB):
                        if jb + 1 < NJB:
                            # prefetch next S while this one exps on ACT
                            sps[jb + 1] = emit_energy(jb + 1)
                        sp = sps.pop(jb)
                        pt = ppool.tile([P, jpb * IW], bf16, tag="p", name="pt")
                        nc.scalar.activation(pt, sp, Exp)
                        for t in range(jpb):
                            jt = jb * jpb + t
                            for s in range(NSL):
                                nc.tensor.matmul(
                                    accs[s],
                                    pt[:, ds(t * IW + s * P, P)],
                                    vW1_sb[:, jt, :],
                                    start=(jb == 0 and t == 0),
                                    stop=(jb == NJB - 1 and t == jpb - 1),
                                )
                    # epilogue: out = acc * (1/denom) + (x^T + b)
                    for s in range(NSL):
                        isl = i * IW + s * P
                        xq = xqpool.tile([P, _C], f32, tag="xq", name="xq")
                        nc.sync.dma_start(out=xq, in_=xqt_b[isl : isl + P, :])
                        rc = epool.tile([P, 1], f32, tag="rc", name="rc")
                        nc.vector.reciprocal(rc, accs[s][:, 256:257])
                        st = epool.tile([P, _C], f32, tag="st", name="st")
                        nc.vector.scalar_tensor_tensor(
                            st, accs[s][:, 0:256], rc[:, :], xq, op0=mult, op1=add
                        )
                        nc.sync.dma_start(out=out_t[isl : isl + P, :], in_=st)

            if reps > 1:
                with tc.For_i(0, reps, 1, hint_engines=(mybir.EngineType.PE,)):
                    compute_body()
            else:
                compute_body()

    nc.compile()
    return nc


def _host_inputs(x, wq, bq, wk, bk, wv, bv, wg, bg, n=_N, nq=_NQ):
    """Per-core input maps (numpy only)."""
    xf = np.ascontiguousarray(x.reshape(_B, _C, n).astype(np.float32))
    W64 = wg.astype(np.float64) @ wv.astype(np.float64)
    b64 = wg.astype(np.float64) @ bv.astype(np.float64) + bg.astype(np.float64)
    Wt = np.ascontiguousarray(W64.T.astype(np.float32))
    bcomb = b64.astype(np.float32)
    wq4t = np.ascontiguousarray(np.tile(wq.T.astype(np.float32), (1, 4)))
    wk4t = np.ascontiguousarray(np.tile(wk.T.astype(np.float32), (1, 4)))
    bq4 = np.ascontiguousarray(np.tile(bq.astype(np.float32), 4)[:, None])
    bk4 = np.ascontiguousarray(np.tile(bk.astype(np.float32), 4)[:, None])
    import ml_dtypes

    ones_col = np.ones((128, n // 128, 8), ml_dtypes.bfloat16)

    halves = n // nq
    in_maps = []
    for core in range(_NCORES):
        b, half = core // halves, core % halves
        off = half * nq
        x_roll = np.ascontiguousarray(np.roll(xf[b], -off, axis=1))
        xqt_b = np.ascontiguousarray(x_roll[:, :nq].T + bcomb[None, :])
        in_maps.append(
            {
                "x_in": x_roll,
                "xqt_b": xqt_b,
                "wq4t": wq4t,
                "wk4t": wk4t,
                "bq4": bq4,
                "bk4": bk4,
                "Wt": Wt,
                "ones_col": ones_col,
            }
        )
    return in_maps


def kernel(x, wq, bq, wk, bk, wv, bv, wg, bg):
    _ensure_path()
    from concourse.bass_utils import run_bass_kernel_spmd

    nc = build_program()
    in_maps = _host_inputs(x, wq, bq, wk, bk, wv, bv, wg, bg)
    core_ids = list(range(_NCORES))
    res = run_bass_kernel_spmd(nc, in_maps, core_ids, trace=TRACE)
    LAST_RUN_INFO["exec_time_ns"] = res.exec_time_ns
    LAST_RUN_INFO["mean_exec_time_ns"] = res.mean_exec_time_ns
    LAST_RUN_INFO["results"] = res

    out = np.empty((_B, _C, _N), np.float32)
    for core in range(_NCORES):
        b, off = core // 2, (core % 2) * _NQ
        out[b, :, off : off + _NQ] = res.results[core]["out_t"].T
    return out.reshape(_B, _C, _H, _W)


# revision 23
# speedup vs baseline: 1.8141x; 1.1040x over previous
"""CrissCrossAttention (full HW-token attention) Trainium2 kernel.

Reference computation (B=4, C=256, H=W=64, N=H*W=4096, CQK=32):
    q = wq@x+bq   [B,32,N]
    k = wk@x+bk   [B,32,N]
    v = wv@x+bv   [B,256,N]
    energy = q^T k      [B,N,N]
    attn = softmax_j(energy)
    out = v @ attn^T    [B,256,N]
    final = x + wg@out + bg

Sharding: 8 cores = 4 batches x 2 query-halves. Each core receives x[b]
rolled so its 2048 query columns are always columns 0:2048 (softmax over
keys is permutation invariant, so rolling keys+values consistently leaves
the result unchanged) -> one identical SPMD program for all cores.

Algebraic folding done on host:
    wg@(v@attn^T)+bg = (wg@wv)@x@attn^T + (wg@bv+bg)    (attn rows sum to 1)
so the kernel only needs W=wg@wv and b=wg@bv+bg, and the g-projection
matmul disappears.

Device layout trick: energy tiles are computed transposed, S_t[j,i]=k^T q,
so the exp'd tile P_t[j,i] feeds the AV matmul directly as the stationary
operand (no transposes anywhere in the main loop). Ones-columns appended
to vW_t produce the softmax denominator inside the same accumulation.
All matmul operands are float32r (FP22-truncated reads, full PE rate).
"""

import sys

import numpy as np

_B, _C, _H, _W = 4, 256, 64, 64
_N = _H * _W  # 4096 key/value positions
_CQK = _C // 8  # 32
_NCORES = 8
_NQ = _N // 2  # 2048 queries per core

# Filled by kernel() for the benefit of test harnesses; never read here.
LAST_RUN_INFO = {}
TRACE = False

_REPO = "/opt/trn_rl_repo"


def _ensure_path():
    if _REPO not in sys.path:
        sys.path.insert(0, _REPO)


def build_program(n=_N, nq=_NQ, jpb=2, reps=1):
    """Build the single-core Bass/Tile program (identical across cores).

    n:    number of key/value positions    (multiple of 512)
    nq:   number of query positions        (multiple of 512)
    jpb:  key j-subtiles (128 keys each) batched per PSUM/exp tile
    reps: repeat the compute body in a HW loop (benchmarking only)
    """
    _ensure_path()
    import concourse.tile as tile
    from concourse import bacc, mybir
    from concourse.bass import ds, ts

    f32 = mybir.dt.float32
    f32r = mybir.dt.float32r
    bf16 = mybir.dt.bfloat16
    Exp = mybir.ActivationFunctionType.Exp
    mult = mybir.AluOpType.mult
    add = mybir.AluOpType.add

    P = 128
    IW = 512  # query-tile width for the energy matmul (N of one MM)
    assert n % (128 * jpb) == 0 and nq % IW == 0
    NJ = n // 128  # j-tiles of 128 keys
    NJB = NJ // jpb  # j batches
    NI = nq // IW  # i-tiles of 512 queries
    NSL = IW // P  # 4 i-slices per i-tile

    nc = bacc.Bacc("TRN2", target_bir_lowering=False, debug=False)

    x_in = nc.dram_tensor("x_in", [_C, n], f32r, kind="ExternalInput")
    xqt_b = nc.dram_tensor("xqt_b", [nq, _C], f32, kind="ExternalInput")
    wq4t = nc.dram_tensor("wq4t", [_C, 128], f32r, kind="ExternalInput")
    wk4t = nc.dram_tensor("wk4t", [_C, 128], f32r, kind="ExternalInput")
    bq4 = nc.dram_tensor("bq4", [128, 1], f32, kind="ExternalInput")
    bk4 = nc.dram_tensor("bk4", [128, 1], f32, kind="ExternalInput")
    x_bf = nc.dram_tensor("x_bf", [_C, n], bf16, kind="ExternalInput")
    Wt = nc.dram_tensor("Wt", [_C, _C], bf16, kind="ExternalInput")
    ones_col = nc.dram_tensor("ones_col", [128, NJ, 8], bf16, kind="ExternalInput")
    out_t = nc.dram_tensor("out_t", [nq, _C], f32, kind="ExternalOutput")

    with tile.TileContext(nc) as tc:
        with (
            tc.tile_pool(name="singles", bufs=1) as singles,
            tc.tile_pool(name="ptile", bufs=3) as ppool,
            tc.tile_pool(name="epi", bufs=4) as epool,
            tc.tile_pool(name="xq", bufs=4) as xqpool,
            tc.tile_pool(name="spsum", bufs=2, space="PSUM") as spool,
            tc.tile_pool(name="accpsum", bufs=4, space="PSUM") as accpool,
        ):
            # ---- persistent SBUF tensors ----
            x_sb = [
                singles.tile([P, n], f32r, tag=f"x{c}", name=f"x_sb{c}")
                for c in range(2)
            ]
            k4_sb = singles.tile([P, n], f32r, tag="k4")
            q4_sb = singles.tile([P, nq], f32r, tag="q4")
            vW1_sb = singles.tile([P, NJ, 264], bf16, tag="vw1")
            wq4_sb = [
                singles.tile([P, 128], f32r, tag=f"wq{c}", name=f"wq4_sb{c}")
                for c in range(2)
            ]
            wk4_sb = [
                singles.tile([P, 128], f32r, tag=f"wk{c}", name=f"wk4_sb{c}")
                for c in range(2)
            ]
            Wt_sb = [
                singles.tile([P, _C], bf16, tag=f"wt{c}", name=f"Wt_sb{c}")
                for c in range(2)
            ]
            xbf_sb = [
                singles.tile([P, n], bf16, tag=f"xb{c}", name=f"xbf_sb{c}")
                for c in range(2)
            ]
            bq4_sb = singles.tile([P, 1], f32, tag="bq")
            bk4_sb = singles.tile([P, 1], f32, tag="bk")
            # residual (x^T + b) staged once; epilogue reads slices
            xqt_sb = singles.tile([P, nq // P, _C], f32, tag="xqt")

            for c in range(2):
                nc.sync.dma_start(out=x_sb[c], in_=x_in[c * P : (c + 1) * P, :])
                nc.sync.dma_start(out=xbf_sb[c], in_=x_bf[c * P : (c + 1) * P, :])
                nc.sync.dma_start(out=wq4_sb[c], in_=wq4t[c * P : (c + 1) * P, :])
                nc.sync.dma_start(out=wk4_sb[c], in_=wk4t[c * P : (c + 1) * P, :])
                nc.sync.dma_start(out=Wt_sb[c], in_=Wt[c * P : (c + 1) * P, :])
            nc.sync.dma_start(out=bq4_sb, in_=bq4[:, :])
            nc.sync.dma_start(out=bk4_sb, in_=bk4[:, :])
            nc.sync.dma_start(
                out=xqt_sb, in_=xqt_b[:, :].rearrange("(t p) c -> p t c", p=P)
            )

            # ones columns -> softmax denominator rides along the AV matmul
            nc.sync.dma_start(out=vW1_sb[:, :, 256:264], in_=ones_col[:, :, :])

            def compute_body():
                # ---- projections ----
                # k (4x replicated over partition groups): k4 = wk4t^T @ x + bk
                for t in range(n // IW):
                    kp = spool.tile([P, IW], f32, tag="s", name="kp")
                    for c in range(2):
                        nc.tensor.matmul(
                            kp,
                            wk4_sb[c][:, :],
                            x_sb[c][:, ts(t, IW)],
                            start=(c == 0),
                            stop=(c == 1),
                        )
                    nc.vector.tensor_scalar_add(k4_sb[:, ts(t, IW)], kp, bk4_sb[:, :])

                # q for our query columns (0:nq of the rolled x)
                for t in range(nq // IW):
                    qp = spool.tile([P, IW], f32, tag="s", name="qp")
                    for c in range(2):
                        nc.tensor.matmul(
                            qp,
                            wq4_sb[c][:, :],
                            x_sb[c][:, ts(t, IW)],
                            start=(c == 0),
                            stop=(c == 1),
                        )
                    nc.vector.tensor_scalar_add(q4_sb[:, ts(t, IW)], qp, bq4_sb[:, :])

                # vW_t[j, c] = (W @ x)^T = x^T @ W^T, per j-tile
                for j in range(NJ):
                    vp = spool.tile([P, _C], f32, tag="s", name="vp")
                    for c in range(2):
                        nc.tensor.matmul(
                            vp,
                            xbf_sb[c][:, ts(j, P)],
                            Wt_sb[c][:, :],
                            start=(c == 0),
                            stop=(c == 1),
                        )
                    nc.vector.tensor_copy(vW1_sb[:, j, 0:256], vp)

                # ---- attention main loop ----
                for i in range(NI):
                    accs = [
                        accpool.tile([P, 264], f32, tag="acc", name="acc")
                        for _ in range(NSL)
                    ]

                    def emit_energy(jb, i=i):
                        # S_t[j, i] = sum_d k[d, j] * q[d, i]  (K = 32); the
                        # jpb j-subtiles go to distinct PE row groups (k/q
                        # are replicated across partition groups for this)
                        sp = spool.tile([P, jpb * IW], f32, tag="s", name="sp")
                        for t in range(jpb):
                            jt = jb * jpb + t
                            nc.tensor.matmul(
                                sp[:, ts(t, IW)],
                                k4_sb[32 * t : 32 * t + _CQK, ts(jt, P)],
                                q4_sb[32 * t : 32 * t + _CQK, ts(i, IW)],
                                start=True,
                                stop=True,
                                tile_position=(32 * t, 0),
                            )
                        return sp

                    sps = {0: emit_energy(0)}
                    for jb in range(NJB):
                        if jb + 1 < NJB:
                            # prefetch next S while this one exps on ACT
                            sps[jb + 1] = emit_energy(jb + 1)
                        sp = sps.pop(jb)
                        pt = ppool.tile([P, jpb * IW], bf16, tag="p", name="pt")
                        nc.scalar.activation(pt, sp, Exp)
                        for t in range(jpb):
                            jt = jb * jpb + t
                            for s in range(NSL):
                                nc.tensor.matmul(
                                    accs[s],
                                    pt[:, ds(t * IW + s * P, P)],
                                    vW1_sb[:, jt, :],
                                    start=(jb == 0 and t == 0),
                                    stop=(jb == NJB - 1 and t == jpb - 1),
                                )
                    # epilogue: out = acc * (1/denom) + (x^T + b)
                    for s in range(NSL):
                        isl = i * IW + s * P
                        rc = epool.tile([P, 1], f32, tag="rc", name="rc")
                        nc.vector.reciprocal(rc, accs[s][:, 256:257])
                        st = epool.tile([P, _C], f32, tag="st", name="st")
                        nc.vector.scalar_tensor_tensor(
                            st,
                            accs[s][:, 0:256],
                            rc[:, :],
                            xqt_sb[:, i * NSL + s, :],
                            op0=mult,
                            op1=add,
                        )
                        nc.sync.dma_start(out=out_t[isl : isl + P, :], in_=st)

            if reps > 1:
                with tc.For_i(0, reps, 1, hint_engines=(mybir.EngineType.PE,)):
                    compute_body()
            else:
                compute_body()

    nc.compile()
    return nc


def _host_inputs(x, wq, bq, wk, bk, wv, bv, wg, bg, n=_N, nq=_NQ):
    """Per-core input maps (numpy only)."""
    xf = np.ascontiguousarray(x.reshape(_B, _C, n).astype(np.float32))
    W64 = wg.astype(np.float64) @ wv.astype(np.float64)
    b64 = wg.astype(np.float64) @ bv.astype(np.float64) + bg.astype(np.float64)
    Wt = np.ascontiguousarray(W64.T.astype(np.float32))
    bcomb = b64.astype(np.float32)
    wq4t = np.ascontiguousarray(np.tile(wq.T.astype(np.float32), (1, 4)))
    wk4t = np.ascontiguousarray(np.tile(wk.T.astype(np.float32), (1, 4)))
    bq4 = np.ascontiguousarray(np.tile(bq.astype(np.float32), 4)[:, None])
    bk4 = np.ascontiguousarray(np.tile(bk.astype(np.float32), 4)[:, None])
    import ml_dtypes

    Wt_bf = Wt.astype(ml_dtypes.bfloat16)
    ones_col = np.ones((128, n // 128, 8), ml_dtypes.bfloat16)

    halves = n // nq
    in_maps = []
    for core in range(_NCORES):
        b, half = core // halves, core % halves
        off = half * nq
        x_roll = np.ascontiguousarray(np.roll(xf[b], -off, axis=1))
        xqt_b = np.ascontiguousarray(x_roll[:, :nq].T + bcomb[None, :])
        in_maps.append(
            {
                "x_in": x_roll,
                "x_bf": x_roll.astype(ml_dtypes.bfloat16),
                "xqt_b": xqt_b,
                "wq4t": wq4t,
                "wk4t": wk4t,
                "bq4": bq4,
                "bk4": bk4,
                "Wt": Wt_bf,
                "ones_col": ones_col,
            }
        )
    return in_maps


def kernel(x, wq, bq, wk, bk, wv, bv, wg, bg):
    _ensure_path()
    from concourse.bass_utils import run_bass_kernel_spmd

    nc = build_program()
    in_maps = _host_inputs(x, wq, bq, wk, bk, wv, bv, wg, bg)
    core_ids = list(range(_NCORES))
    res = run_bass_kernel_spmd(nc, in_maps, core_ids, trace=TRACE)
    LAST_RUN_INFO["exec_time_ns"] = res.exec_time_ns
    LAST_RUN_INFO["mean_exec_time_ns"] = res.mean_exec_time_ns
    LAST_RUN_INFO["results"] = res

    out = np.empty((_B, _C, _N), np.float32)
    for core in range(_NCORES):
        b, off = core // 2, (core % 2) * _NQ
        out[b, :, off : off + _NQ] = res.results[core]["out_t"].T
    return out.reshape(_B, _C, _H, _W)
